# revision 3
# baseline (speedup 1.0000x reference)
"""Bahdanau attention on 8 TRN2 NeuronCores.

Reference math (H=2048, S=4096):
    enc     = encoder_outputs[..., 0]                      # (S, H)
    wh      = W_h @ decoder_hidden                         # (H,)
    we      = enc @ W_e.T                                  # (S, H)
    energy  = tanh(we + wh + b)                            # (S, H)
    scores  = energy @ v[0]                                # (S,)
    attn    = softmax(scores)
    context = attn @ enc                                   # (H,)

Sharding: S across 8 cores (512 rows each). W_h sharded by rows (256 each,
all-gathered after the local matvec). W_e/v/b/decoder_hidden replicated.
Per core, everything is computed in "energy-transposed" orientation
(h on partitions, s on free dim) so the +wh+b bias and the tanh fuse into a
single ScalarE activation reading PSUM, and the v-contraction is a PE matmul.

The contraction dim (k over H) must sit on SBUF partitions for the PE, so
both W_e and enc are transposed on-chip with cheap identity matmuls
(out = lhsT.T @ I), which run at full PE rate, unlike transpose-mode.

The softmax + context all-reduce is collapsed into ONE AllGather: each core
ships (local scores, local max m, local sum Z, unnormalized context partial
cpart = exp(scores-m) @ enc_local); every core then combines
    M = max_i m_i,  Zg = sum_i Z_i e^{m_i-M},
    context = sum_i (e^{m_i-M}/Zg) cpart_i,  attn = exp(scores-M)/Zg
redundantly and writes identical full outputs.
"""

import numpy as np
import ml_dtypes

H = 2048
S = 4096
NCORES = 8
P = 128
SL = S // NCORES          # 512 local encoder steps
WHL = H // NCORES         # 256 local W_h rows
KT = H // P               # 16 contraction tiles
HT = H // P               # 16 h tiles
ST = SL // P              # 4 local s tiles
NG = SL + H + 4           # gather payload: scores | cpart | m | Z | pad2

_CACHE = {}


def _build():
    import concourse.bacc as bacc
    import concourse.tile as tile
    import concourse.mybir as mybir

    dt = mybir.dt
    AF = mybir.ActivationFunctionType
    AX = mybir.AxisListType

    nc = bacc.Bacc(None, target_bir_lowering=False, num_devices=NCORES)

    enc_d = nc.declare_dram_parameter("enc", [SL, H], dt.float32, isOutput=False)
    we_d = nc.declare_dram_parameter("w_e", [H, H], dt.float32, isOutput=False)
    wh_d = nc.declare_dram_parameter("w_h", [WHL, H], dt.float32, isOutput=False)
    dh_d = nc.declare_dram_parameter("dh", [H], dt.float32, isOutput=False)
    v_d = nc.declare_dram_parameter("vv", [H], dt.float32, isOutput=False)
    b_d = nc.declare_dram_parameter("bb", [H], dt.float32, isOutput=False)
    id_d = nc.declare_dram_parameter("ident", [P, P], dt.bfloat16, isOutput=False)
    on_d = nc.declare_dram_parameter("ones", [1, P], dt.float32, isOutput=False)
    ctx_d = nc.declare_dram_parameter("ctx", [H], dt.float32, isOutput=True)
    attn_d = nc.declare_dram_parameter("attn", [S], dt.float32, isOutput=True)

    with tile.TileContext(nc) as tc:
        with (
            tc.tile_pool(name="const", bufs=1) as const_p,
            tc.tile_pool(name="big", bufs=1) as big_p,
            tc.tile_pool(name="wen", bufs=3) as wen_p,
            tc.tile_pool(name="energy", bufs=3) as en_p,
            tc.tile_pool(name="small", bufs=1) as small_p,
            tc.tile_pool(name="tps", bufs=2, space="PSUM") as tp_p,
            tc.tile_pool(name="pe", bufs=2, space="PSUM") as pe_p,
            tc.tile_pool(name="scps", bufs=1, space="PSUM") as sc_p,
            tc.tile_pool(name="tailps", bufs=1, space="PSUM") as tail_p,
            tc.tile_pool(name="dram", bufs=1, space="DRAM") as dram_p,
        ):
            # ---- constants / vectors ----
            id_sb = const_p.tile([P, P], dt.bfloat16)
            nc.sync.dma_start(id_sb[:], id_d[:, :])
            ones_sb = const_p.tile([1, P], dt.float32)
            nc.sync.dma_start(ones_sb[:], on_d[:, :])
            # k-major vector tiles: elem (p, j) = x[j*128 + p]
            dh_sb = const_p.tile([P, KT], dt.bfloat16)
            nc.gpsimd.dma_start(dh_sb[:], dh_d[:].rearrange("(j p) -> p j", p=P))
            v_sb = const_p.tile([P, HT], dt.bfloat16)
            nc.gpsimd.dma_start(v_sb[:], v_d[:].rearrange("(j p) -> p j", p=P))
            b_sb = const_p.tile([P, HT], dt.float32)
            nc.sync.dma_start(b_sb[:], b_d[:].rearrange("(j p) -> p j", p=P))

            # ---- W_h shard -> local wh column, all-gather ----
            whn = const_p.tile([P, 2 * H], dt.bfloat16)
            for t2 in range(2):
                nc.gpsimd.dma_start(
                    whn[:, t2 * H:(t2 + 1) * H], wh_d[t2 * P:(t2 + 1) * P, :]
                )
            whT = const_p.tile([P, 2 * H], dt.bfloat16)  # blocks (t2*16+kt)*128
            for t2 in range(2):
                for g in range(4):
                    ps = tp_p.tile([P, 512], dt.float32, tag="tps")
                    for j in range(4):
                        kt = g * 4 + j
                        nc.tensor.matmul(
                            ps[:, j * P:(j + 1) * P],
                            whn[:, t2 * H + kt * P: t2 * H + (kt + 1) * P],
                            id_sb[:],
                            start=True, stop=True,
                        )
                    eng = nc.vector if g % 2 == 0 else nc.scalar
                    dst = whT[:, (t2 * 16 + g * 4) * P:(t2 * 16 + g * 4 + 4) * P]
                    if eng is nc.vector:
                        eng.tensor_copy(dst, ps[:])
                    else:
                        eng.copy(dst, ps[:])
            wh_ps = sc_p.tile([P, 2], dt.float32, tag="whps")
            for t2 in range(2):
                for kt in range(KT):
                    nc.tensor.matmul(
                        wh_ps[:, t2:t2 + 1],
                        whT[:, (t2 * 16 + kt) * P:(t2 * 16 + kt + 1) * P],
                        dh_sb[:, kt:kt + 1],
                        start=(kt == 0), stop=(kt == KT - 1),
                    )
            whloc = small_p.tile([P, 2], dt.float32)
            nc.vector.tensor_copy(whloc[:], wh_ps[:])
            wh_in = dram_p.tile([2 * P], dt.float32)
            nc.sync.dma_start(wh_in[:].rearrange("(t p) -> p t", p=P), whloc[:])
            wh_all = dram_p.tile([H], dt.float32)
            nc.gpsimd.collective_compute(
                "AllGather", mybir.AluOpType.bypass,
                replica_groups=[list(range(NCORES))],
                ins=[wh_in[:].opt()], outs=[wh_all[:].opt()],
            )
            whb = const_p.tile([P, HT], dt.float32)  # wh + b, per-partition bias
            whg = small_p.tile([P, HT], dt.float32)
            nc.sync.dma_start(whg[:], wh_all[:].rearrange("(j p) -> p j", p=P))
            nc.vector.tensor_add(whb[:], whg[:], b_sb[:])

            # ---- enc natural (bf16) + transposed ----
            encn = big_p.tile([P, ST * H], dt.bfloat16)  # block st: [:, st*H:+H]
            for st in range(ST):
                nc.gpsimd.dma_start(
                    encn[:, st * H:(st + 1) * H], enc_d[st * P:(st + 1) * P, :]
                )
            encT = big_p.tile([P, KT * SL], dt.bfloat16)  # block kt: [:, kt*SL:+SL]
            for kt in range(KT):
                ps = tp_p.tile([P, 512], dt.float32, tag="tps")
                for st in range(ST):
                    nc.tensor.matmul(
                        ps[:, st * P:(st + 1) * P],
                        encn[:, st * H + kt * P: st * H + (kt + 1) * P],
                        id_sb[:],
                        start=True, stop=True,
                    )
                eng = nc.vector if kt % 2 == 0 else nc.scalar
                dst = encT[:, kt * SL:(kt + 1) * SL]
                if eng is nc.vector:
                    eng.tensor_copy(dst, ps[:])
                else:
                    eng.copy(dst, ps[:])

            # ---- W_e transpose + main matmul, software-pipelined over ht ----
            weT = big_p.tile([P, KT * H], dt.bfloat16)  # lhsT blk (kt,ht): [:, kt*H+ht*P]
            sc_ps = sc_p.tile([1, SL], dt.float32, tag="scores")

            def transpose_we(ht):
                wen_t = wen_p.tile([P, H], dt.bfloat16, tag="wen")
                nc.gpsimd.dma_start(wen_t[:], we_d[ht * P:(ht + 1) * P, :])
                for g in range(4):
                    ps = tp_p.tile([P, 512], dt.float32, tag="tps")
                    for j in range(4):
                        kt = g * 4 + j
                        nc.tensor.matmul(
                            ps[:, j * P:(j + 1) * P],
                            wen_t[:, kt * P:(kt + 1) * P],
                            id_sb[:],
                            start=True, stop=True,
                        )
                    dst = weT[:].rearrange("p (k c) -> p k c", c=H)[
                        :, g * 4:(g + 1) * 4, ht * P:(ht + 1) * P
                    ]
                    src = ps[:].rearrange("p (k c) -> p k c", k=4)
                    if g % 2 == 0:
                        nc.vector.tensor_copy(dst, src)
                    else:
                        nc.scalar.copy(dst, src)

            def main_mm(ht):
                pe_ps = pe_p.tile([P, SL], dt.float32, tag="pe")
                for kt in range(KT):
                    nc.tensor.matmul(
                        pe_ps[:],
                        weT[:, kt * H + ht * P: kt * H + ht * P + P],
                        encT[:, kt * SL:(kt + 1) * SL],
                        start=(kt == 0), stop=(kt == KT - 1),
                    )
                et = en_p.tile([P, SL], dt.bfloat16, tag="energy")
                nc.scalar.activation(
                    et[:], pe_ps[:], AF.Tanh, bias=whb[:, ht:ht + 1], scale=1.0
                )
                nc.tensor.matmul(
                    sc_ps[:], v_sb[:, ht:ht + 1], et[:],
                    start=(ht == 0), stop=(ht == HT - 1),
                )

            transpose_we(0)
            for ht in range(HT):
                if ht + 1 < HT:
                    transpose_we(ht + 1)
                main_mm(ht)

            # ---- local softmax stats + unnormalized context partial ----
            gpay = small_p.tile([1, NG], dt.float32)  # scores | cpart | m | Z | pad
            nc.vector.tensor_copy(gpay[0:1, 0:SL], sc_ps[:])
            nc.gpsimd.memset(gpay[0:1, SL + H + 2:NG], 0.0)
            m_ap = gpay[0:1, SL + H:SL + H + 1]
            z_ap = gpay[0:1, SL + H + 1:SL + H + 2]
            nc.vector.reduce_max(m_ap, gpay[0:1, 0:SL], axis=AX.X)
            negm = small_p.tile([1, 1], dt.float32)
            nc.vector.tensor_scalar_mul(negm[:], m_ap, -1.0)
            p_row = small_p.tile([1, SL], dt.float32)
            nc.scalar.activation(
                p_row[:], gpay[0:1, 0:SL], AF.Exp,
                bias=negm[:], scale=1.0, accum_out=z_ap,
            )
            p_bf = small_p.tile([1, SL], dt.bfloat16)
            nc.vector.tensor_copy(p_bf[:], p_row[:])
            pc_ps = tail_p.tile([P, ST], dt.float32, tag="tail")
            for st in range(ST):
                nc.tensor.matmul(
                    pc_ps[:, st:st + 1],
                    p_bf[0:1, st * P:(st + 1) * P],
                    id_sb[0:1, 0:1],
                    start=True, stop=True,
                )
            p_col = small_p.tile([P, ST], dt.bfloat16)
            nc.vector.tensor_copy(p_col[:], pc_ps[:])
            for hc in range(4):
                cp_ps = tail_p.tile([1, 512], dt.float32, tag="tail")
                for st in range(ST):
                    nc.tensor.matmul(
                        cp_ps[:],
                        p_col[:, st:st + 1],
                        encn[:, st * H + hc * 512: st * H + (hc + 1) * 512],
                        start=(st == 0), stop=(st == ST - 1),
                    )
                nc.vector.tensor_copy(
                    gpay[0:1, SL + hc * 512:SL + (hc + 1) * 512], cp_ps[:]
                )

            # ---- the one AllGather ----
            g_in = dram_p.tile([NG], dt.float32)
            nc.sync.dma_start(g_in[:], gpay[:])
            g_out = dram_p.tile([NCORES * NG], dt.float32)
            nc.gpsimd.collective_compute(
                "AllGather", mybir.AluOpType.bypass,
                replica_groups=[list(range(NCORES))],
                ins=[g_in[:].opt()], outs=[g_out[:].opt()],
            )
            g_view = g_out[:].rearrange("(r c) -> r c", c=NG)

            # ---- global combine (identical on every core) ----
            mz = small_p.tile([1, 2 * NCORES], dt.float32)  # r-major (m, Z) pairs
            nc.sync.dma_start(mz[:], g_view[:, SL + H:SL + H + 2])
            mzv = mz[:].rearrange("o (r c) -> o r c", c=2)
            m_row = mzv[:, :, 0:1]
            z_row = mzv[:, :, 1:2]
            M = small_p.tile([1, 1], dt.float32)
            nc.vector.reduce_max(M[:], m_row, axis=AX.XY)
            negM = small_p.tile([1, 1], dt.float32)
            nc.vector.tensor_scalar_mul(negM[:], M[:], -1.0)
            w_row = small_p.tile([1, NCORES], dt.float32)
            nc.scalar.activation(
                w_row[:], m_row, AF.Exp, bias=negM[:], scale=1.0
            )
            zz = small_p.tile([1, NCORES], dt.float32)
            nc.vector.tensor_mul(zz[:], w_row[:], z_row)
            Zg = small_p.tile([1, 1], dt.float32)
            nc.vector.reduce_sum(Zg[:], zz[:], axis=AX.X)
            rZg = small_p.tile([1, 1], dt.float32)
            nc.vector.reciprocal(rZg[:], Zg[:])
            w8 = small_p.tile([1, NCORES], dt.float32)
            nc.vector.tensor_scalar_mul(w8[:], w_row[:], rZg[:])

            # broadcast negM / rZg to 128 partitions via K=1 f32 matmul
            bc_in = small_p.tile([1, 2], dt.float32)
            nc.vector.tensor_copy(bc_in[0:1, 0:1], negM[:])
            nc.vector.tensor_copy(bc_in[0:1, 1:2], rZg[:])
            bc_ps = tail_p.tile([P, 2], dt.float32, tag="tail")
            nc.tensor.matmul(bc_ps[:], ones_sb[:], bc_in[:], start=True, stop=True)
            bc = small_p.tile([P, 2], dt.float32)
            nc.vector.tensor_copy(bc[:], bc_ps[:])

            # w8 as a column (8 partitions) via K=1 f32 matmul
            w8c_ps = tail_p.tile([NCORES, 1], dt.float32, tag="tail")
            nc.tensor.matmul(
                w8c_ps[:], w8[:], ones_sb[0:1, 0:1], start=True, stop=True
            )
            w8c = small_p.tile([NCORES, 1], dt.float32)
            nc.vector.tensor_copy(w8c[:], w8c_ps[:])

            # context = sum_i w8[i] * cpart_i  (f32 matmul, K=8)
            cpm = small_p.tile([NCORES, H], dt.float32)
            nc.sync.dma_start(cpm[:], g_view[:, SL:SL + H])
            ctx_sb = small_p.tile([1, H], dt.float32)
            for hc in range(4):
                cx_ps = tail_p.tile([1, 512], dt.float32, tag="tail")
                nc.tensor.matmul(
                    cx_ps[:], w8c[:], cpm[:, hc * 512:(hc + 1) * 512],
                    start=True, stop=True,
                )
                nc.vector.tensor_copy(ctx_sb[0:1, hc * 512:(hc + 1) * 512], cx_ps[:])
            nc.sync.dma_start(ctx_d[:], ctx_sb[:])

            # attn = exp(scores - M) / Zg over all S, laid out [128, 32]
            scr = small_p.tile([P, S // P], dt.float32)
            for r in range(NCORES):
                nc.sync.dma_start(
                    scr[r * 16:(r + 1) * 16, :],
                    g_out[r * NG:r * NG + SL].rearrange("(p j) -> p j", j=S // P),
                )
            p_all = small_p.tile([P, S // P], dt.float32)
            nc.scalar.activation(
                p_all[:], scr[:], AF.Exp, bias=bc[:, 0:1], scale=1.0
            )
            attn_sb = small_p.tile([P, S // P], dt.float32)
            nc.vector.tensor_scalar_mul(attn_sb[:], p_all[:], bc[:, 1:2])
            nc.sync.dma_start(
                attn_d[:].rearrange("(p j) -> p j", j=S // P), attn_sb[:]
            )

    nc.finalize()
    return nc


def _get_nc():
    if "nc" not in _CACHE:
        _CACHE["nc"] = _build()
    return _CACHE["nc"]


def _execute(inputs, trace=False):
    from concourse.bass_utils import run_bass_kernel_spmd

    nc = _get_nc()
    dh = np.ascontiguousarray(
        np.asarray(inputs["decoder_hidden"], dtype=np.float32).reshape(H)
    )
    enc = np.ascontiguousarray(
        np.asarray(inputs["encoder_outputs"], dtype=np.float32).reshape(S, H)
    )
    w_h = np.ascontiguousarray(np.asarray(inputs["W_h"], dtype=np.float32))
    w_e = np.ascontiguousarray(np.asarray(inputs["W_e"], dtype=np.float32))
    bb = np.ascontiguousarray(
        np.asarray(inputs["b"], dtype=np.float32).reshape(H)
    )
    vv = np.ascontiguousarray(
        np.asarray(inputs["v"], dtype=np.float32).reshape(H)
    )
    ident = np.eye(P, dtype=ml_dtypes.bfloat16)
    ones = np.ones((1, P), dtype=np.float32)

    in_maps = []
    for i in range(NCORES):
        in_maps.append({
            "enc": enc[i * SL:(i + 1) * SL],
            "w_e": w_e,
            "w_h": w_h[i * WHL:(i + 1) * WHL],
            "dh": dh,
            "vv": vv,
            "bb": bb,
            "ident": ident,
            "ones": ones,
        })

    res = run_bass_kernel_spmd(nc, in_maps, core_ids=list(range(NCORES)),
                               trace=trace)
    out = res.results[0]
    context = np.asarray(out["ctx"], dtype=np.float32).reshape(H, 1)
    attn = np.asarray(out["attn"], dtype=np.float32).reshape(S)
    return (context, attn), res


def kernel(**inputs):
    (context, attn), _ = _execute(inputs, trace=False)
    return context, attn


# revision 8
# speedup vs baseline: 1.0685x; 1.0685x over previous
"""Bahdanau attention on 8 TRN2 NeuronCores.

Reference math (H=2048, S=4096):
    enc     = encoder_outputs[..., 0]                      # (S, H)
    wh      = W_h @ decoder_hidden                         # (H,)
    we      = enc @ W_e.T                                  # (S, H)
    energy  = tanh(we + wh + b)                            # (S, H)
    scores  = energy @ v[0]                                # (S,)
    attn    = softmax(scores)
    context = attn @ enc                                   # (H,)

Sharding: S across 8 cores (512 rows each). W_h sharded by rows (256 each,
all-gathered after the local matvec). W_e/v/b/decoder_hidden replicated.
Per core, everything is computed in "energy-transposed" orientation
(h on partitions, s on free dim) so the +wh+b bias and the tanh fuse into a
single ScalarE activation reading PSUM, and the v-contraction is a PE matmul.

The contraction dim (k over H) must sit on SBUF partitions for the PE, so
both W_e and enc are transposed on-chip with cheap identity matmuls
(out = lhsT.T @ I), which run at full PE rate, unlike transpose-mode.

The softmax + context all-reduce is collapsed into ONE AllGather: each core
ships (local scores, local max m, local sum Z, unnormalized context partial
cpart = exp(scores-m) @ enc_local); every core then combines
    M = max_i m_i,  Zg = sum_i Z_i e^{m_i-M},
    context = sum_i (e^{m_i-M}/Zg) cpart_i,  attn = exp(scores-M)/Zg
redundantly and writes identical full outputs.
"""

import numpy as np
import ml_dtypes

H = 2048
S = 4096
NCORES = 8
P = 128
SL = S // NCORES          # 512 local encoder steps
WHL = H // NCORES         # 256 local W_h rows
KT = H // P               # 16 contraction tiles
HT = H // P               # 16 h tiles
ST = SL // P              # 4 local s tiles
NG = SL + H + 4           # gather payload: scores | cpart | m | Z | pad2

_CACHE = {}


def _build():
    import concourse.bacc as bacc
    import concourse.tile as tile
    import concourse.mybir as mybir

    dt = mybir.dt
    AF = mybir.ActivationFunctionType
    AX = mybir.AxisListType

    nc = bacc.Bacc(None, target_bir_lowering=False, num_devices=NCORES)

    enc_d = nc.declare_dram_parameter("enc", [SL, H], dt.float32, isOutput=False)
    we_d = nc.declare_dram_parameter("w_e", [H, H], dt.float32, isOutput=False)
    wh_d = nc.declare_dram_parameter("w_h", [WHL, H], dt.float32, isOutput=False)
    dh_d = nc.declare_dram_parameter("dh", [H], dt.float32, isOutput=False)
    v_d = nc.declare_dram_parameter("vv", [H], dt.float32, isOutput=False)
    b_d = nc.declare_dram_parameter("bb", [H], dt.float32, isOutput=False)
    id_d = nc.declare_dram_parameter("ident", [P, P], dt.bfloat16, isOutput=False)
    on_d = nc.declare_dram_parameter("ones", [1, P], dt.float32, isOutput=False)
    ctx_d = nc.declare_dram_parameter("ctx", [H], dt.float32, isOutput=True)
    attn_d = nc.declare_dram_parameter("attn", [S], dt.float32, isOutput=True)

    with tile.TileContext(nc) as tc:
        with (
            tc.tile_pool(name="const", bufs=1) as const_p,
            tc.tile_pool(name="big", bufs=1) as big_p,
            tc.tile_pool(name="wen", bufs=3) as wen_p,
            tc.tile_pool(name="energy", bufs=3) as en_p,
            tc.tile_pool(name="small", bufs=1) as small_p,
            tc.tile_pool(name="tps", bufs=3, space="PSUM") as tp_p,
            tc.tile_pool(name="pe", bufs=2, space="PSUM") as pe_p,
            tc.tile_pool(name="scps", bufs=1, space="PSUM") as sc_p,
            tc.tile_pool(name="tailps", bufs=1, space="PSUM") as tail_p,
            tc.tile_pool(name="dram", bufs=1, space="DRAM") as dram_p,
        ):
            # ---- constants / vectors ----
            id_sb = const_p.tile([P, P], dt.bfloat16)
            nc.sync.dma_start(id_sb[:], id_d[:, :])
            ones_sb = const_p.tile([1, P], dt.float32)
            nc.sync.dma_start(ones_sb[:], on_d[:, :])
            # k-major vector tiles: elem (p, j) = x[j*128 + p]
            dh_sb = const_p.tile([P, KT], dt.bfloat16)
            nc.gpsimd.dma_start(dh_sb[:], dh_d[:].rearrange("(j p) -> p j", p=P))
            v_sb = const_p.tile([P, HT], dt.bfloat16)
            nc.gpsimd.dma_start(v_sb[:], v_d[:].rearrange("(j p) -> p j", p=P))
            b_sb = const_p.tile([P, HT], dt.float32)
            nc.sync.dma_start(b_sb[:], b_d[:].rearrange("(j p) -> p j", p=P))

            # ---- enc natural (bf16) + transposed ----
            encn = big_p.tile([P, ST * H], dt.bfloat16)  # block st: [:, st*H:+H]
            for st in range(ST):
                nc.gpsimd.dma_start(
                    encn[:, st * H:(st + 1) * H], enc_d[st * P:(st + 1) * P, :]
                )
            wen_tiles = {}

            def load_wen(ht):
                wen_t = wen_p.tile([P, H], dt.bfloat16, tag="wen")
                nc.gpsimd.dma_start(wen_t[:], we_d[ht * P:(ht + 1) * P, :])
                wen_tiles[ht] = wen_t

            # ---- W_h shard -> local wh column, all-gather ----
            whn = const_p.tile([P, 2 * H], dt.bfloat16)
            for t2 in range(2):
                nc.gpsimd.dma_start(
                    whn[:, t2 * H:(t2 + 1) * H], wh_d[t2 * P:(t2 + 1) * P, :]
                )
            whT = const_p.tile([P, 2 * H], dt.bfloat16)  # blocks (t2*16+kt)*128
            for t2 in range(2):
                for g in range(4):
                    ps = tp_p.tile([P, 512], dt.float32, tag="tps")
                    for j in range(4):
                        kt = g * 4 + j
                        nc.tensor.matmul(
                            ps[:, j * P:(j + 1) * P],
                            whn[:, t2 * H + kt * P: t2 * H + (kt + 1) * P],
                            id_sb[:],
                            start=True, stop=True,
                        )
                    eng = nc.vector if g % 2 == 0 else nc.scalar
                    dst = whT[:, (t2 * 16 + g * 4) * P:(t2 * 16 + g * 4 + 4) * P]
                    if eng is nc.vector:
                        eng.tensor_copy(dst, ps[:])
                    else:
                        eng.copy(dst, ps[:])
            wh_ps = sc_p.tile([P, 2], dt.float32, tag="whps")
            for t2 in range(2):
                for kt in range(KT):
                    nc.tensor.matmul(
                        wh_ps[:, t2:t2 + 1],
                        whT[:, (t2 * 16 + kt) * P:(t2 * 16 + kt + 1) * P],
                        dh_sb[:, kt:kt + 1],
                        start=(kt == 0), stop=(kt == KT - 1),
                    )
            whloc = small_p.tile([P, 2], dt.float32)
            nc.vector.tensor_copy(whloc[:], wh_ps[:])
            wh_in = dram_p.tile([2 * P], dt.float32)
            nc.sync.dma_start(wh_in[:].rearrange("(t p) -> p t", p=P), whloc[:])
            wh_all = dram_p.tile([H], dt.float32)
            load_wen(0)
            load_wen(1)
            nc.gpsimd.collective_compute(
                "AllGather", mybir.AluOpType.bypass,
                replica_groups=[list(range(NCORES))],
                ins=[wh_in[:].opt()], outs=[wh_all[:].opt()],
            )
            whb = const_p.tile([P, HT], dt.float32)  # wh + b, per-partition bias
            whg = small_p.tile([P, HT], dt.float32)
            nc.sync.dma_start(whg[:], wh_all[:].rearrange("(j p) -> p j", p=P))
            nc.vector.tensor_add(whb[:], whg[:], b_sb[:])

            encT = big_p.tile([P, KT * SL], dt.bfloat16)  # block kt: [:, kt*SL:+SL]
            for kt in range(KT):
                ps = tp_p.tile([P, 512], dt.float32, tag="tps")
                for st in range(ST):
                    nc.tensor.matmul(
                        ps[:, st * P:(st + 1) * P],
                        encn[:, st * H + kt * P: st * H + (kt + 1) * P],
                        id_sb[:],
                        start=True, stop=True,
                    )
                eng = nc.vector if kt % 2 == 0 else nc.scalar
                dst = encT[:, kt * SL:(kt + 1) * SL]
                if eng is nc.vector:
                    eng.tensor_copy(dst, ps[:])
                else:
                    eng.copy(dst, ps[:])

            # ---- W_e transpose + main matmul, software-pipelined over ht ----
            weT = big_p.tile([P, KT * H], dt.bfloat16)  # lhsT blk (kt,ht): [:, kt*H+ht*P]
            raw = big_p.tile([P, HT * SL], dt.bfloat16)  # we_eT, tanh deferred
            sc_ps = sc_p.tile([1, SL], dt.float32, tag="scores")

            def transpose_we(ht):
                if ht not in wen_tiles:
                    load_wen(ht)
                wen_t = wen_tiles.pop(ht)
                for g in range(4):
                    ps = tp_p.tile([P, 512], dt.float32, tag="tps")
                    for j in range(4):
                        kt = g * 4 + j
                        nc.tensor.matmul(
                            ps[:, j * P:(j + 1) * P],
                            wen_t[:, kt * P:(kt + 1) * P],
                            id_sb[:],
                            start=True, stop=True,
                        )
                    dst = weT[:].rearrange("p (k c) -> p k c", c=H)[
                        :, g * 4:(g + 1) * 4, ht * P:(ht + 1) * P
                    ]
                    src = ps[:].rearrange("p (k c) -> p k c", k=4)
                    if g % 2 == 0:
                        nc.vector.tensor_copy(dst, src)
                    else:
                        nc.scalar.copy(dst, src)

            def main_mm(ht):
                pe_ps = pe_p.tile([P, SL], dt.float32, tag="pe")
                for kt in range(KT):
                    nc.tensor.matmul(
                        pe_ps[:],
                        weT[:, kt * H + ht * P: kt * H + ht * P + P],
                        encT[:, kt * SL:(kt + 1) * SL],
                        start=(kt == 0), stop=(kt == KT - 1),
                    )
                # Drain raw we_eT; tanh is applied later once the wh
                # all-gather (slow first-collective barrier) has landed.
                dst = raw[:, ht * SL:(ht + 1) * SL]
                if ht % 2 == 0:
                    nc.vector.tensor_copy(dst, pe_ps[:])
                else:
                    nc.scalar.copy(dst, pe_ps[:])

            transpose_we(0)
            for ht in range(HT):
                if ht + 1 < HT:
                    transpose_we(ht + 1)
                main_mm(ht)

            # ---- deferred tanh(+wh+b) and v-contraction ----
            for ht in range(HT):
                et = en_p.tile([P, SL], dt.bfloat16, tag="energy")
                nc.scalar.activation(
                    et[:], raw[:, ht * SL:(ht + 1) * SL], AF.Tanh,
                    bias=whb[:, ht:ht + 1], scale=1.0,
                )
                nc.tensor.matmul(
                    sc_ps[:], v_sb[:, ht:ht + 1], et[:],
                    start=(ht == 0), stop=(ht == HT - 1),
                )

            # ---- local softmax stats + unnormalized context partial ----
            gpay = small_p.tile([1, NG], dt.float32)  # scores | cpart | m | Z | pad
            nc.vector.tensor_copy(gpay[0:1, 0:SL], sc_ps[:])
            nc.gpsimd.memset(gpay[0:1, SL + H + 2:NG], 0.0)
            m_ap = gpay[0:1, SL + H:SL + H + 1]
            z_ap = gpay[0:1, SL + H + 1:SL + H + 2]
            nc.vector.reduce_max(m_ap, gpay[0:1, 0:SL], axis=AX.X)
            negm = small_p.tile([1, 1], dt.float32)
            nc.vector.tensor_scalar_mul(negm[:], m_ap, -1.0)
            p_row = small_p.tile([1, SL], dt.float32)
            nc.scalar.activation(
                p_row[:], gpay[0:1, 0:SL], AF.Exp,
                bias=negm[:], scale=1.0, accum_out=z_ap,
            )
            p_bf = small_p.tile([1, SL], dt.bfloat16)
            nc.vector.tensor_copy(p_bf[:], p_row[:])
            pc_ps = tail_p.tile([P, ST], dt.float32, tag="tail")
            for st in range(ST):
                nc.tensor.matmul(
                    pc_ps[:, st:st + 1],
                    p_bf[0:1, st * P:(st + 1) * P],
                    id_sb[0:1, 0:1],
                    start=True, stop=True,
                )
            p_col = small_p.tile([P, ST], dt.bfloat16)
            nc.vector.tensor_copy(p_col[:], pc_ps[:])
            for hc in range(4):
                cp_ps = tail_p.tile([1, 512], dt.float32, tag="tail")
                for st in range(ST):
                    nc.tensor.matmul(
                        cp_ps[:],
                        p_col[:, st:st + 1],
                        encn[:, st * H + hc * 512: st * H + (hc + 1) * 512],
                        start=(st == 0), stop=(st == ST - 1),
                    )
                nc.vector.tensor_copy(
                    gpay[0:1, SL + hc * 512:SL + (hc + 1) * 512], cp_ps[:]
                )

            # ---- the one AllGather ----
            g_in = dram_p.tile([NG], dt.float32)
            nc.sync.dma_start(g_in[:], gpay[:])
            g_out = dram_p.tile([NCORES * NG], dt.float32)
            nc.gpsimd.collective_compute(
                "AllGather", mybir.AluOpType.bypass,
                replica_groups=[list(range(NCORES))],
                ins=[g_in[:].opt()], outs=[g_out[:].opt()],
            )
            g_view = g_out[:].rearrange("(r c) -> r c", c=NG)

            # ---- global combine (identical on every core) ----
            mz = small_p.tile([1, 2 * NCORES], dt.float32)  # r-major (m, Z) pairs
            nc.sync.dma_start(mz[:], g_view[:, SL + H:SL + H + 2])
            mzv = mz[:].rearrange("o (r c) -> o r c", c=2)
            m_row = mzv[:, :, 0:1]
            z_row = mzv[:, :, 1:2]
            M = small_p.tile([1, 1], dt.float32)
            nc.vector.reduce_max(M[:], m_row, axis=AX.XY)
            negM = small_p.tile([1, 1], dt.float32)
            nc.vector.tensor_scalar_mul(negM[:], M[:], -1.0)
            w_row = small_p.tile([1, NCORES], dt.float32)
            nc.scalar.activation(
                w_row[:], m_row, AF.Exp, bias=negM[:], scale=1.0
            )
            zz = small_p.tile([1, NCORES], dt.float32)
            nc.vector.tensor_mul(zz[:], w_row[:], z_row)
            Zg = small_p.tile([1, 1], dt.float32)
            nc.vector.reduce_sum(Zg[:], zz[:], axis=AX.X)
            rZg = small_p.tile([1, 1], dt.float32)
            nc.vector.reciprocal(rZg[:], Zg[:])
            w8 = small_p.tile([1, NCORES], dt.float32)
            nc.vector.tensor_scalar_mul(w8[:], w_row[:], rZg[:])

            # broadcast negM / rZg to 128 partitions via K=1 f32 matmul
            bc_in = small_p.tile([1, 2], dt.float32)
            nc.vector.tensor_copy(bc_in[0:1, 0:1], negM[:])
            nc.vector.tensor_copy(bc_in[0:1, 1:2], rZg[:])
            bc_ps = tail_p.tile([P, 2], dt.float32, tag="tail")
            nc.tensor.matmul(bc_ps[:], ones_sb[:], bc_in[:], start=True, stop=True)
            bc = small_p.tile([P, 2], dt.float32)
            nc.vector.tensor_copy(bc[:], bc_ps[:])

            # w8 as a column (8 partitions) via K=1 f32 matmul
            w8c_ps = tail_p.tile([NCORES, 1], dt.float32, tag="tail")
            nc.tensor.matmul(
                w8c_ps[:], w8[:], ones_sb[0:1, 0:1], start=True, stop=True
            )
            w8c = small_p.tile([NCORES, 1], dt.float32)
            nc.vector.tensor_copy(w8c[:], w8c_ps[:])

            # context = sum_i w8[i] * cpart_i  (f32 matmul, K=8)
            cpm = small_p.tile([NCORES, H], dt.float32)
            nc.sync.dma_start(cpm[:], g_view[:, SL:SL + H])
            ctx_sb = small_p.tile([1, H], dt.float32)
            for hc in range(4):
                cx_ps = tail_p.tile([1, 512], dt.float32, tag="tail")
                nc.tensor.matmul(
                    cx_ps[:], w8c[:], cpm[:, hc * 512:(hc + 1) * 512],
                    start=True, stop=True,
                )
                nc.vector.tensor_copy(ctx_sb[0:1, hc * 512:(hc + 1) * 512], cx_ps[:])
            nc.sync.dma_start(ctx_d[:], ctx_sb[:])

            # attn = exp(scores - M) / Zg over all S, laid out [128, 32]
            scr = small_p.tile([P, S // P], dt.float32)
            nc.sync.dma_start(
                scr[:, :],
                g_view[:, 0:SL].rearrange("r (p j) -> r p j", j=S // P),
            )
            p_all = small_p.tile([P, S // P], dt.float32)
            nc.scalar.activation(
                p_all[:], scr[:], AF.Exp, bias=bc[:, 0:1], scale=1.0
            )
            attn_sb = small_p.tile([P, S // P], dt.float32)
            nc.vector.tensor_scalar_mul(attn_sb[:], p_all[:], bc[:, 1:2])
            nc.sync.dma_start(
                attn_d[:].rearrange("(p j) -> p j", j=S // P), attn_sb[:]
            )

    nc.finalize()
    return nc


def _get_nc():
    if "nc" not in _CACHE:
        _CACHE["nc"] = _build()
    return _CACHE["nc"]


def _execute(inputs, trace=False):
    from concourse.bass_utils import run_bass_kernel_spmd

    nc = _get_nc()
    dh = np.ascontiguousarray(
        np.asarray(inputs["decoder_hidden"], dtype=np.float32).reshape(H)
    )
    enc = np.ascontiguousarray(
        np.asarray(inputs["encoder_outputs"], dtype=np.float32).reshape(S, H)
    )
    w_h = np.ascontiguousarray(np.asarray(inputs["W_h"], dtype=np.float32))
    w_e = np.ascontiguousarray(np.asarray(inputs["W_e"], dtype=np.float32))
    bb = np.ascontiguousarray(
        np.asarray(inputs["b"], dtype=np.float32).reshape(H)
    )
    vv = np.ascontiguousarray(
        np.asarray(inputs["v"], dtype=np.float32).reshape(H)
    )
    ident = np.eye(P, dtype=ml_dtypes.bfloat16)
    ones = np.ones((1, P), dtype=np.float32)

    in_maps = []
    for i in range(NCORES):
        in_maps.append({
            "enc": enc[i * SL:(i + 1) * SL],
            "w_e": w_e,
            "w_h": w_h[i * WHL:(i + 1) * WHL],
            "dh": dh,
            "vv": vv,
            "bb": bb,
            "ident": ident,
            "ones": ones,
        })

    res = run_bass_kernel_spmd(nc, in_maps, core_ids=list(range(NCORES)),
                               trace=trace)
    out = res.results[0]
    context = np.asarray(out["ctx"], dtype=np.float32).reshape(H, 1)
    attn = np.asarray(out["attn"], dtype=np.float32).reshape(S)
    return (context, attn), res


def kernel(**inputs):
    (context, attn), _ = _execute(inputs, trace=False)
    return context, attn


# revision 9
# speedup vs baseline: 1.0870x; 1.0172x over previous
"""Bahdanau attention on 8 TRN2 NeuronCores.

Reference math (H=2048, S=4096):
    enc     = encoder_outputs[..., 0]                      # (S, H)
    wh      = W_h @ decoder_hidden                         # (H,)
    we      = enc @ W_e.T                                  # (S, H)
    energy  = tanh(we + wh + b)                            # (S, H)
    scores  = energy @ v[0]                                # (S,)
    attn    = softmax(scores)
    context = attn @ enc                                   # (H,)

Sharding: S across 8 cores (512 rows each). W_h sharded by rows (256 each,
all-gathered after the local matvec). W_e/v/b/decoder_hidden replicated.
Per core, everything is computed in "energy-transposed" orientation
(h on partitions, s on free dim) so the +wh+b bias and the tanh fuse into a
single ScalarE activation reading PSUM, and the v-contraction is a PE matmul.

The contraction dim (k over H) must sit on SBUF partitions for the PE, so
both W_e and enc are transposed on-chip with cheap identity matmuls
(out = lhsT.T @ I), which run at full PE rate, unlike transpose-mode.

The softmax + context all-reduce is collapsed into ONE AllGather: each core
ships (local scores, local max m, local sum Z, unnormalized context partial
cpart = exp(scores-m) @ enc_local); every core then combines
    M = max_i m_i,  Zg = sum_i Z_i e^{m_i-M},
    context = sum_i (e^{m_i-M}/Zg) cpart_i,  attn = exp(scores-M)/Zg
redundantly and writes identical full outputs.
"""

import numpy as np
import ml_dtypes

H = 2048
S = 4096
NCORES = 8
P = 128
SL = S // NCORES          # 512 local encoder steps
WHL = H // NCORES         # 256 local W_h rows
KT = H // P               # 16 contraction tiles
HT = H // P               # 16 h tiles
ST = SL // P              # 4 local s tiles
NG = SL + H + 4           # gather payload: scores | cpart | m | Z | pad2

_CACHE = {}


def _build():
    import concourse.bacc as bacc
    import concourse.tile as tile
    import concourse.mybir as mybir

    dt = mybir.dt
    AF = mybir.ActivationFunctionType
    AX = mybir.AxisListType

    nc = bacc.Bacc(None, target_bir_lowering=False, num_devices=NCORES)

    enc_d = nc.declare_dram_parameter("enc", [SL, H], dt.float32, isOutput=False)
    we_d = nc.declare_dram_parameter("w_e", [H, H], dt.float32, isOutput=False)
    wh_d = nc.declare_dram_parameter("w_h", [WHL, H], dt.float32, isOutput=False)
    dh_d = nc.declare_dram_parameter("dh", [H], dt.float32, isOutput=False)
    v_d = nc.declare_dram_parameter("vv", [H], dt.float32, isOutput=False)
    b_d = nc.declare_dram_parameter("bb", [H], dt.float32, isOutput=False)
    id_d = nc.declare_dram_parameter("ident", [P, P], dt.bfloat16, isOutput=False)
    on_d = nc.declare_dram_parameter("ones", [1, P], dt.float32, isOutput=False)
    ctx_d = nc.declare_dram_parameter("ctx", [H], dt.float32, isOutput=True)
    attn_d = nc.declare_dram_parameter("attn", [S], dt.float32, isOutput=True)

    with tile.TileContext(nc) as tc:
        with (
            tc.tile_pool(name="const", bufs=1) as const_p,
            tc.tile_pool(name="big", bufs=1) as big_p,
            tc.tile_pool(name="wen", bufs=3) as wen_p,
            tc.tile_pool(name="energy", bufs=3) as en_p,
            tc.tile_pool(name="small", bufs=1) as small_p,
            tc.tile_pool(name="tps", bufs=3, space="PSUM") as tp_p,
            tc.tile_pool(name="pe", bufs=2, space="PSUM") as pe_p,
            tc.tile_pool(name="scps", bufs=1, space="PSUM") as sc_p,
            tc.tile_pool(name="tailps", bufs=1, space="PSUM") as tail_p,
            tc.tile_pool(name="dram", bufs=1, space="DRAM") as dram_p,
        ):
            # ---- constants / vectors ----
            id_sb = const_p.tile([P, P], dt.bfloat16)
            nc.sync.dma_start(id_sb[:], id_d[:, :])
            ones_sb = const_p.tile([1, P], dt.float32)
            nc.sync.dma_start(ones_sb[:], on_d[:, :])
            # k-major vector tiles: elem (p, j) = x[j*128 + p]
            dh_sb = const_p.tile([P, KT], dt.bfloat16)
            nc.gpsimd.dma_start(dh_sb[:], dh_d[:].rearrange("(j p) -> p j", p=P))
            v_sb = const_p.tile([P, HT], dt.bfloat16)
            nc.gpsimd.dma_start(v_sb[:], v_d[:].rearrange("(j p) -> p j", p=P))
            b_sb = const_p.tile([P, HT], dt.float32)
            nc.sync.dma_start(b_sb[:], b_d[:].rearrange("(j p) -> p j", p=P))

            wen_tiles = {}

            def load_wen(ht):
                wen_t = wen_p.tile([P, H], dt.bfloat16, tag="wen")
                nc.gpsimd.dma_start(wen_t[:], we_d[ht * P:(ht + 1) * P, :])
                wen_tiles[ht] = wen_t

            # ---- W_h shard -> local wh column, all-gather ----
            whn = const_p.tile([P, 2 * H], dt.bfloat16)
            for t2 in range(2):
                nc.gpsimd.dma_start(
                    whn[:, t2 * H:(t2 + 1) * H], wh_d[t2 * P:(t2 + 1) * P, :]
                )
            # ---- enc natural (bf16) ----
            encn = big_p.tile([P, ST * H], dt.bfloat16)  # block st: [:, st*H:+H]
            for st in range(ST):
                nc.gpsimd.dma_start(
                    encn[:, st * H:(st + 1) * H], enc_d[st * P:(st + 1) * P, :]
                )
            whT = const_p.tile([P, 2 * H], dt.bfloat16)  # blocks (t2*16+kt)*128
            for t2 in range(2):
                for g in range(4):
                    ps = tp_p.tile([P, 512], dt.float32, tag="tps")
                    for j in range(4):
                        kt = g * 4 + j
                        nc.tensor.matmul(
                            ps[:, j * P:(j + 1) * P],
                            whn[:, t2 * H + kt * P: t2 * H + (kt + 1) * P],
                            id_sb[:],
                            start=True, stop=True,
                        )
                    eng = nc.vector if g % 2 == 0 else nc.scalar
                    dst = whT[:, (t2 * 16 + g * 4) * P:(t2 * 16 + g * 4 + 4) * P]
                    if eng is nc.vector:
                        eng.tensor_copy(dst, ps[:])
                    else:
                        eng.copy(dst, ps[:])
            wh_ps = sc_p.tile([P, 2], dt.float32, tag="whps")
            for t2 in range(2):
                for kt in range(KT):
                    nc.tensor.matmul(
                        wh_ps[:, t2:t2 + 1],
                        whT[:, (t2 * 16 + kt) * P:(t2 * 16 + kt + 1) * P],
                        dh_sb[:, kt:kt + 1],
                        start=(kt == 0), stop=(kt == KT - 1),
                    )
            whloc = small_p.tile([P, 2], dt.float32)
            nc.vector.tensor_copy(whloc[:], wh_ps[:])
            wh_in = dram_p.tile([2 * P], dt.float32)
            nc.sync.dma_start(wh_in[:].rearrange("(t p) -> p t", p=P), whloc[:])
            wh_all = dram_p.tile([H], dt.float32)
            load_wen(0)
            load_wen(1)
            nc.gpsimd.collective_compute(
                "AllGather", mybir.AluOpType.bypass,
                replica_groups=[list(range(NCORES))],
                ins=[wh_in[:].opt()], outs=[wh_all[:].opt()],
            )
            encT = big_p.tile([P, KT * SL], dt.bfloat16)  # block kt: [:, kt*SL:+SL]
            for st in range(ST):
                for g in range(4):
                    ps = tp_p.tile([P, 512], dt.float32, tag="tps")
                    for j in range(4):
                        kt = g * 4 + j
                        nc.tensor.matmul(
                            ps[:, j * P:(j + 1) * P],
                            encn[:, st * H + kt * P: st * H + (kt + 1) * P],
                            id_sb[:],
                            start=True, stop=True,
                        )
                    dst = encT[:].rearrange("p (k c) -> p k c", c=SL)[
                        :, g * 4:(g + 1) * 4, st * P:(st + 1) * P
                    ]
                    srcv = ps[:].rearrange("p (k c) -> p k c", k=4)
                    if (st * 4 + g) % 2 == 0:
                        nc.vector.tensor_copy(dst, srcv)
                    else:
                        nc.scalar.copy(dst, srcv)

            # ---- W_e transpose + main matmul, software-pipelined over ht ----
            weT = big_p.tile([P, KT * H], dt.bfloat16)  # lhsT blk (kt,ht): [:, kt*H+ht*P]
            raw = big_p.tile([P, HT * SL], dt.bfloat16)  # we_eT, tanh deferred
            sc_ps = sc_p.tile([1, SL], dt.float32, tag="scores")

            def transpose_we(ht):
                if ht not in wen_tiles:
                    load_wen(ht)
                wen_t = wen_tiles.pop(ht)
                for g in range(4):
                    ps = tp_p.tile([P, 512], dt.float32, tag="tps")
                    for j in range(4):
                        kt = g * 4 + j
                        nc.tensor.matmul(
                            ps[:, j * P:(j + 1) * P],
                            wen_t[:, kt * P:(kt + 1) * P],
                            id_sb[:],
                            start=True, stop=True,
                        )
                    dst = weT[:].rearrange("p (k c) -> p k c", c=H)[
                        :, g * 4:(g + 1) * 4, ht * P:(ht + 1) * P
                    ]
                    src = ps[:].rearrange("p (k c) -> p k c", k=4)
                    if g % 2 == 0:
                        nc.vector.tensor_copy(dst, src)
                    else:
                        nc.scalar.copy(dst, src)

            def main_mm(ht):
                pe_ps = pe_p.tile([P, SL], dt.float32, tag="pe")
                for kt in range(KT):
                    nc.tensor.matmul(
                        pe_ps[:],
                        weT[:, kt * H + ht * P: kt * H + ht * P + P],
                        encT[:, kt * SL:(kt + 1) * SL],
                        start=(kt == 0), stop=(kt == KT - 1),
                    )
                # Drain raw we_eT; tanh is applied later once the wh
                # all-gather (slow first-collective barrier) has landed.
                dst = raw[:, ht * SL:(ht + 1) * SL]
                if ht % 2 == 0:
                    nc.vector.tensor_copy(dst, pe_ps[:])
                else:
                    nc.scalar.copy(dst, pe_ps[:])

            transpose_we(0)
            for ht in range(HT):
                if ht + 1 < HT:
                    transpose_we(ht + 1)
                main_mm(ht)

            # ---- fetch gathered wh late (hides first-collective barrier) ----
            whb = const_p.tile([P, HT], dt.float32)  # wh + b, per-partition bias
            whg = small_p.tile([P, HT], dt.float32)
            nc.sync.dma_start(whg[:], wh_all[:].rearrange("(j p) -> p j", p=P))
            nc.vector.tensor_add(whb[:], whg[:], b_sb[:])

            # ---- deferred tanh(+wh+b) and v-contraction ----
            for ht in range(HT):
                et = en_p.tile([P, SL], dt.bfloat16, tag="energy")
                nc.scalar.activation(
                    et[:], raw[:, ht * SL:(ht + 1) * SL], AF.Tanh,
                    bias=whb[:, ht:ht + 1], scale=1.0,
                )
                nc.tensor.matmul(
                    sc_ps[:], v_sb[:, ht:ht + 1], et[:],
                    start=(ht == 0), stop=(ht == HT - 1),
                )

            # ---- local softmax stats + unnormalized context partial ----
            gpay = small_p.tile([1, NG], dt.float32)  # scores | cpart | m | Z | pad
            nc.vector.tensor_copy(gpay[0:1, 0:SL], sc_ps[:])
            nc.gpsimd.memset(gpay[0:1, SL + H + 2:NG], 0.0)
            m_ap = gpay[0:1, SL + H:SL + H + 1]
            z_ap = gpay[0:1, SL + H + 1:SL + H + 2]
            nc.vector.reduce_max(m_ap, gpay[0:1, 0:SL], axis=AX.X)
            negm = small_p.tile([1, 1], dt.float32)
            nc.vector.tensor_scalar_mul(negm[:], m_ap, -1.0)
            p_row = small_p.tile([1, SL], dt.float32)
            nc.scalar.activation(
                p_row[:], gpay[0:1, 0:SL], AF.Exp,
                bias=negm[:], scale=1.0, accum_out=z_ap,
            )
            p_bf = small_p.tile([1, SL], dt.bfloat16)
            nc.vector.tensor_copy(p_bf[:], p_row[:])
            pc_ps = tail_p.tile([P, ST], dt.float32, tag="tail")
            for st in range(ST):
                nc.tensor.matmul(
                    pc_ps[:, st:st + 1],
                    p_bf[0:1, st * P:(st + 1) * P],
                    id_sb[0:1, 0:1],
                    start=True, stop=True,
                )
            p_col = small_p.tile([P, ST], dt.bfloat16)
            nc.vector.tensor_copy(p_col[:], pc_ps[:])
            for hc in range(4):
                cp_ps = tail_p.tile([1, 512], dt.float32, tag="tail")
                for st in range(ST):
                    nc.tensor.matmul(
                        cp_ps[:],
                        p_col[:, st:st + 1],
                        encn[:, st * H + hc * 512: st * H + (hc + 1) * 512],
                        start=(st == 0), stop=(st == ST - 1),
                    )
                nc.vector.tensor_copy(
                    gpay[0:1, SL + hc * 512:SL + (hc + 1) * 512], cp_ps[:]
                )

            # ---- the one AllGather ----
            g_in = dram_p.tile([NG], dt.float32)
            nc.sync.dma_start(g_in[:], gpay[:])
            g_out = dram_p.tile([NCORES * NG], dt.float32)
            nc.gpsimd.collective_compute(
                "AllGather", mybir.AluOpType.bypass,
                replica_groups=[list(range(NCORES))],
                ins=[g_in[:].opt()], outs=[g_out[:].opt()],
            )
            g_view = g_out[:].rearrange("(r c) -> r c", c=NG)

            # ---- global combine (identical on every core) ----
            mz = small_p.tile([1, 2 * NCORES], dt.float32)  # r-major (m, Z) pairs
            nc.sync.dma_start(mz[:], g_view[:, SL + H:SL + H + 2])
            mzv = mz[:].rearrange("o (r c) -> o r c", c=2)
            m_row = mzv[:, :, 0:1]
            z_row = mzv[:, :, 1:2]
            M = small_p.tile([1, 1], dt.float32)
            nc.vector.reduce_max(M[:], m_row, axis=AX.XY)
            negM = small_p.tile([1, 1], dt.float32)
            nc.vector.tensor_scalar_mul(negM[:], M[:], -1.0)
            w_row = small_p.tile([1, NCORES], dt.float32)
            nc.scalar.activation(
                w_row[:], m_row, AF.Exp, bias=negM[:], scale=1.0
            )
            zz = small_p.tile([1, NCORES], dt.float32)
            nc.vector.tensor_mul(zz[:], w_row[:], z_row)
            Zg = small_p.tile([1, 1], dt.float32)
            nc.vector.reduce_sum(Zg[:], zz[:], axis=AX.X)
            rZg = small_p.tile([1, 1], dt.float32)
            nc.vector.reciprocal(rZg[:], Zg[:])
            w8 = small_p.tile([1, NCORES], dt.float32)
            nc.vector.tensor_scalar_mul(w8[:], w_row[:], rZg[:])

            # broadcast negM / rZg to 128 partitions via K=1 f32 matmul
            bc_in = small_p.tile([1, 2], dt.float32)
            nc.vector.tensor_copy(bc_in[0:1, 0:1], negM[:])
            nc.vector.tensor_copy(bc_in[0:1, 1:2], rZg[:])
            bc_ps = tail_p.tile([P, 2], dt.float32, tag="tail")
            nc.tensor.matmul(bc_ps[:], ones_sb[:], bc_in[:], start=True, stop=True)
            bc = small_p.tile([P, 2], dt.float32)
            nc.vector.tensor_copy(bc[:], bc_ps[:])

            # w8 as a column (8 partitions) via K=1 f32 matmul
            w8c_ps = tail_p.tile([NCORES, 1], dt.float32, tag="tail")
            nc.tensor.matmul(
                w8c_ps[:], w8[:], ones_sb[0:1, 0:1], start=True, stop=True
            )
            w8c = small_p.tile([NCORES, 1], dt.float32)
            nc.vector.tensor_copy(w8c[:], w8c_ps[:])

            # context = sum_i w8[i] * cpart_i  (f32 matmul, K=8)
            cpm = small_p.tile([NCORES, H], dt.float32)
            nc.sync.dma_start(cpm[:], g_view[:, SL:SL + H])
            ctx_sb = small_p.tile([1, H], dt.float32)
            for hc in range(4):
                cx_ps = tail_p.tile([1, 512], dt.float32, tag="tail")
                nc.tensor.matmul(
                    cx_ps[:], w8c[:], cpm[:, hc * 512:(hc + 1) * 512],
                    start=True, stop=True,
                )
                nc.vector.tensor_copy(ctx_sb[0:1, hc * 512:(hc + 1) * 512], cx_ps[:])
            nc.sync.dma_start(ctx_d[:], ctx_sb[:])

            # attn = exp(scores - M) / Zg over all S, laid out [128, 32]
            scr = small_p.tile([P, S // P], dt.float32)
            nc.sync.dma_start(
                scr[:, :],
                g_view[:, 0:SL].rearrange("r (p j) -> r p j", j=S // P),
            )
            p_all = small_p.tile([P, S // P], dt.float32)
            nc.scalar.activation(
                p_all[:], scr[:], AF.Exp, bias=bc[:, 0:1], scale=1.0
            )
            attn_sb = small_p.tile([P, S // P], dt.float32)
            nc.vector.tensor_scalar_mul(attn_sb[:], p_all[:], bc[:, 1:2])
            nc.sync.dma_start(
                attn_d[:].rearrange("(p j) -> p j", j=S // P), attn_sb[:]
            )

    nc.finalize()
    return nc


def _get_nc():
    if "nc" not in _CACHE:
        _CACHE["nc"] = _build()
    return _CACHE["nc"]


def _execute(inputs, trace=False):
    from concourse.bass_utils import run_bass_kernel_spmd

    nc = _get_nc()
    dh = np.ascontiguousarray(
        np.asarray(inputs["decoder_hidden"], dtype=np.float32).reshape(H)
    )
    enc = np.ascontiguousarray(
        np.asarray(inputs["encoder_outputs"], dtype=np.float32).reshape(S, H)
    )
    w_h = np.ascontiguousarray(np.asarray(inputs["W_h"], dtype=np.float32))
    w_e = np.ascontiguousarray(np.asarray(inputs["W_e"], dtype=np.float32))
    bb = np.ascontiguousarray(
        np.asarray(inputs["b"], dtype=np.float32).reshape(H)
    )
    vv = np.ascontiguousarray(
        np.asarray(inputs["v"], dtype=np.float32).reshape(H)
    )
    ident = np.eye(P, dtype=ml_dtypes.bfloat16)
    ones = np.ones((1, P), dtype=np.float32)

    in_maps = []
    for i in range(NCORES):
        in_maps.append({
            "enc": enc[i * SL:(i + 1) * SL],
            "w_e": w_e,
            "w_h": w_h[i * WHL:(i + 1) * WHL],
            "dh": dh,
            "vv": vv,
            "bb": bb,
            "ident": ident,
            "ones": ones,
        })

    res = run_bass_kernel_spmd(nc, in_maps, core_ids=list(range(NCORES)),
                               trace=trace)
    out = res.results[0]
    context = np.asarray(out["ctx"], dtype=np.float32).reshape(H, 1)
    attn = np.asarray(out["attn"], dtype=np.float32).reshape(S)
    return (context, attn), res


def kernel(**inputs):
    (context, attn), _ = _execute(inputs, trace=False)
    return context, attn


# revision 10
# speedup vs baseline: 1.1195x; 1.0299x over previous
"""Bahdanau attention on 8 TRN2 NeuronCores.

Reference math (H=2048, S=4096):
    enc     = encoder_outputs[..., 0]                      # (S, H)
    wh      = W_h @ decoder_hidden                         # (H,)
    we      = enc @ W_e.T                                  # (S, H)
    energy  = tanh(we + wh + b)                            # (S, H)
    scores  = energy @ v[0]                                # (S,)
    attn    = softmax(scores)
    context = attn @ enc                                   # (H,)

Sharding: S across 8 cores (512 rows each). W_h sharded by rows (256 each,
all-gathered after the local matvec). W_e/v/b/decoder_hidden replicated.
Per core, everything is computed in "energy-transposed" orientation
(h on partitions, s on free dim) so the +wh+b bias and the tanh fuse into a
single ScalarE activation reading PSUM, and the v-contraction is a PE matmul.

The contraction dim (k over H) must sit on SBUF partitions for the PE, so
both W_e and enc are transposed on-chip with cheap identity matmuls
(out = lhsT.T @ I), which run at full PE rate, unlike transpose-mode.

The softmax + context all-reduce is collapsed into ONE AllGather: each core
ships (local scores, local max m, local sum Z, unnormalized context partial
cpart = exp(scores-m) @ enc_local); every core then combines
    M = max_i m_i,  Zg = sum_i Z_i e^{m_i-M},
    context = sum_i (e^{m_i-M}/Zg) cpart_i,  attn = exp(scores-M)/Zg
redundantly and writes identical full outputs.
"""

import numpy as np
import ml_dtypes

H = 2048
S = 4096
NCORES = 8
P = 128
SL = S // NCORES          # 512 local encoder steps
WHL = H // NCORES         # 256 local W_h rows
KT = H // P               # 16 contraction tiles
HT = H // P               # 16 h tiles
ST = SL // P              # 4 local s tiles
NG = SL + H + 4           # gather payload: scores | cpart | m | Z | pad2

_CACHE = {}


def _build():
    import concourse.bacc as bacc
    import concourse.tile as tile
    import concourse.mybir as mybir

    dt = mybir.dt
    AF = mybir.ActivationFunctionType
    AX = mybir.AxisListType

    nc = bacc.Bacc(None, target_bir_lowering=False, num_devices=NCORES)

    enc_d = nc.declare_dram_parameter("enc", [SL, H], dt.float32, isOutput=False)
    we_d = nc.declare_dram_parameter("w_e", [H, H], dt.float32, isOutput=False)
    wh_d = nc.declare_dram_parameter("w_h", [WHL, H], dt.float32, isOutput=False)
    dh_d = nc.declare_dram_parameter("dh", [H], dt.float32, isOutput=False)
    v_d = nc.declare_dram_parameter("vv", [H], dt.float32, isOutput=False)
    b_d = nc.declare_dram_parameter("bb", [H], dt.float32, isOutput=False)
    id_d = nc.declare_dram_parameter("ident", [P, P], dt.bfloat16, isOutput=False)
    on_d = nc.declare_dram_parameter("ones", [1, P], dt.float32, isOutput=False)
    ctx_d = nc.declare_dram_parameter("ctx", [H], dt.float32, isOutput=True)
    attn_d = nc.declare_dram_parameter("attn", [S], dt.float32, isOutput=True)

    with tile.TileContext(nc) as tc:
        with (
            tc.tile_pool(name="const", bufs=1) as const_p,
            tc.tile_pool(name="big", bufs=1) as big_p,
            tc.tile_pool(name="wen", bufs=16) as wen_p,
            tc.tile_pool(name="energy", bufs=3) as en_p,
            tc.tile_pool(name="wet", bufs=3) as wet_p,
            tc.tile_pool(name="small", bufs=1) as small_p,
            tc.tile_pool(name="tps", bufs=3, space="PSUM") as tp_p,
            tc.tile_pool(name="pe", bufs=2, space="PSUM") as pe_p,
            tc.tile_pool(name="scps", bufs=1, space="PSUM") as sc_p,
            tc.tile_pool(name="tailps", bufs=1, space="PSUM") as tail_p,
            tc.tile_pool(name="dram", bufs=1, space="DRAM") as dram_p,
        ):
            # ---- constants / vectors ----
            id_sb = const_p.tile([P, P], dt.bfloat16)
            nc.sync.dma_start(id_sb[:], id_d[:, :])
            ones_sb = const_p.tile([1, P], dt.float32)
            nc.sync.dma_start(ones_sb[:], on_d[:, :])
            # k-major vector tiles: elem (p, j) = x[j*128 + p]
            dh_sb = const_p.tile([P, KT], dt.bfloat16)
            nc.gpsimd.dma_start(dh_sb[:], dh_d[:].rearrange("(j p) -> p j", p=P))
            v_sb = const_p.tile([P, HT], dt.bfloat16)
            nc.gpsimd.dma_start(v_sb[:], v_d[:].rearrange("(j p) -> p j", p=P))
            b_sb = const_p.tile([P, HT], dt.float32)
            nc.sync.dma_start(b_sb[:], b_d[:].rearrange("(j p) -> p j", p=P))

            wen_tiles = {}

            def load_wen(ht):
                wen_t = wen_p.tile([P, H], dt.bfloat16, tag="wen")
                nc.gpsimd.dma_start(wen_t[:], we_d[ht * P:(ht + 1) * P, :])
                wen_tiles[ht] = wen_t

            # ---- W_h shard -> local wh column, all-gather ----
            whn_t = []
            for t2 in range(2):
                t = const_p.tile([P, H], dt.bfloat16, tag=f"whn{t2}")
                nc.gpsimd.dma_start(t[:], wh_d[t2 * P:(t2 + 1) * P, :])
                whn_t.append(t)
            # ---- enc natural (bf16), one tile per s-block ----
            encn_t = []
            for st in range(ST):
                t = big_p.tile([P, H], dt.bfloat16, tag=f"encn{st}")
                nc.gpsimd.dma_start(t[:], enc_d[st * P:(st + 1) * P, :])
                encn_t.append(t)
            whT = const_p.tile([P, 2 * H], dt.bfloat16)  # blocks (t2*16+kt)*128
            for t2 in range(2):
                for g in range(4):
                    ps = tp_p.tile([P, 512], dt.float32, tag="tps")
                    for j in range(4):
                        kt = g * 4 + j
                        nc.tensor.matmul(
                            ps[:, j * P:(j + 1) * P],
                            whn_t[t2][:, kt * P:(kt + 1) * P],
                            id_sb[:],
                            start=True, stop=True,
                        )
                    eng = nc.vector if g % 2 == 0 else nc.scalar
                    dst = whT[:, (t2 * 16 + g * 4) * P:(t2 * 16 + g * 4 + 4) * P]
                    if eng is nc.vector:
                        eng.tensor_copy(dst, ps[:])
                    else:
                        eng.copy(dst, ps[:])
            wh_ps = sc_p.tile([P, 2], dt.float32, tag="whps")
            for t2 in range(2):
                for kt in range(KT):
                    nc.tensor.matmul(
                        wh_ps[:, t2:t2 + 1],
                        whT[:, (t2 * 16 + kt) * P:(t2 * 16 + kt + 1) * P],
                        dh_sb[:, kt:kt + 1],
                        start=(kt == 0), stop=(kt == KT - 1),
                    )
            whloc = small_p.tile([P, 2], dt.float32)
            nc.vector.tensor_copy(whloc[:], wh_ps[:])
            wh_in = dram_p.tile([2 * P], dt.float32)
            nc.sync.dma_start(wh_in[:].rearrange("(t p) -> p t", p=P), whloc[:])
            wh_all = dram_p.tile([H], dt.float32)
            for _ht in range(HT):
                load_wen(_ht)
            nc.gpsimd.collective_compute(
                "AllGather", mybir.AluOpType.bypass,
                replica_groups=[list(range(NCORES))],
                ins=[wh_in[:].opt()], outs=[wh_all[:].opt()],
            )
            encT = big_p.tile([P, KT * SL], dt.bfloat16)  # block kt: [:, kt*SL:+SL]
            for st in range(ST):
                for g in range(4):
                    ps = tp_p.tile([P, 512], dt.float32, tag="tps")
                    for j in range(4):
                        kt = g * 4 + j
                        nc.tensor.matmul(
                            ps[:, j * P:(j + 1) * P],
                            encn_t[st][:, kt * P:(kt + 1) * P],
                            id_sb[:],
                            start=True, stop=True,
                        )
                    dst = encT[:].rearrange("p (k c) -> p k c", c=SL)[
                        :, g * 4:(g + 1) * 4, st * P:(st + 1) * P
                    ]
                    srcv = ps[:].rearrange("p (k c) -> p k c", k=4)
                    if (st * 4 + g) % 2 == 0:
                        nc.vector.tensor_copy(dst, srcv)
                    else:
                        nc.scalar.copy(dst, srcv)

            # ---- W_e transpose + main matmul, software-pipelined over ht ----
            weT_t = {}   # per-ht [P, KT*P] lhsT tiles, blk kt at [:, kt*P:+P]
            raw = big_p.tile([P, HT * SL], dt.bfloat16)  # we_eT, tanh deferred
            sc_ps = sc_p.tile([1, SL], dt.float32, tag="scores")

            def transpose_we(ht):
                if ht not in wen_tiles:
                    load_wen(ht)
                wen_t = wen_tiles.pop(ht)
                wt = wet_p.tile([P, KT * P], dt.bfloat16, tag="wet")
                weT_t[ht] = wt
                for g in range(4):
                    ps = tp_p.tile([P, 512], dt.float32, tag="tps")
                    for j in range(4):
                        kt = g * 4 + j
                        nc.tensor.matmul(
                            ps[:, j * P:(j + 1) * P],
                            wen_t[:, kt * P:(kt + 1) * P],
                            id_sb[:],
                            start=True, stop=True,
                        )
                    if g % 2 == 0:
                        nc.vector.tensor_copy(wt[:, g * 512:(g + 1) * 512], ps[:])
                    else:
                        nc.scalar.copy(wt[:, g * 512:(g + 1) * 512], ps[:])

            def main_mm(ht):
                pe_ps = pe_p.tile([P, SL], dt.float32, tag="pe")
                wt = weT_t.pop(ht)
                for kt in range(KT):
                    nc.tensor.matmul(
                        pe_ps[:],
                        wt[:, kt * P:(kt + 1) * P],
                        encT[:, kt * SL:(kt + 1) * SL],
                        start=(kt == 0), stop=(kt == KT - 1),
                    )
                # Drain raw we_eT; tanh is applied later once the wh
                # all-gather (slow first-collective barrier) has landed.
                dst = raw[:, ht * SL:(ht + 1) * SL]
                if ht % 2 == 0:
                    nc.vector.tensor_copy(dst, pe_ps[:])
                else:
                    nc.scalar.copy(dst, pe_ps[:])

            transpose_we(0)
            for ht in range(HT):
                if ht + 1 < HT:
                    transpose_we(ht + 1)
                main_mm(ht)

            # ---- fetch gathered wh late (hides first-collective barrier) ----
            whb = const_p.tile([P, HT], dt.float32)  # wh + b, per-partition bias
            whg = small_p.tile([P, HT], dt.float32)
            nc.sync.dma_start(whg[:], wh_all[:].rearrange("(j p) -> p j", p=P))
            nc.vector.tensor_add(whb[:], whg[:], b_sb[:])

            # ---- deferred tanh(+wh+b) and v-contraction ----
            for ht in range(HT):
                et = en_p.tile([P, SL], dt.bfloat16, tag="energy")
                nc.scalar.activation(
                    et[:], raw[:, ht * SL:(ht + 1) * SL], AF.Tanh,
                    bias=whb[:, ht:ht + 1], scale=1.0,
                )
                nc.tensor.matmul(
                    sc_ps[:], v_sb[:, ht:ht + 1], et[:],
                    start=(ht == 0), stop=(ht == HT - 1),
                )

            # ---- local softmax stats + unnormalized context partial ----
            gpay = small_p.tile([1, NG], dt.float32)  # scores | cpart | m | Z | pad
            nc.vector.tensor_copy(gpay[0:1, 0:SL], sc_ps[:])
            nc.gpsimd.memset(gpay[0:1, SL + H + 2:NG], 0.0)
            m_ap = gpay[0:1, SL + H:SL + H + 1]
            z_ap = gpay[0:1, SL + H + 1:SL + H + 2]
            nc.vector.reduce_max(m_ap, gpay[0:1, 0:SL], axis=AX.X)
            negm = small_p.tile([1, 1], dt.float32)
            nc.vector.tensor_scalar_mul(negm[:], m_ap, -1.0)
            p_row = small_p.tile([1, SL], dt.float32)
            nc.scalar.activation(
                p_row[:], gpay[0:1, 0:SL], AF.Exp,
                bias=negm[:], scale=1.0, accum_out=z_ap,
            )
            p_bf = small_p.tile([1, SL], dt.bfloat16)
            nc.vector.tensor_copy(p_bf[:], p_row[:])
            pc_ps = tail_p.tile([P, ST], dt.float32, tag="tail")
            for st in range(ST):
                nc.tensor.matmul(
                    pc_ps[:, st:st + 1],
                    p_bf[0:1, st * P:(st + 1) * P],
                    id_sb[0:1, 0:1],
                    start=True, stop=True,
                )
            p_col = small_p.tile([P, ST], dt.bfloat16)
            nc.vector.tensor_copy(p_col[:], pc_ps[:])
            for hc in range(4):
                cp_ps = tail_p.tile([1, 512], dt.float32, tag="tail")
                for st in range(ST):
                    nc.tensor.matmul(
                        cp_ps[:],
                        p_col[:, st:st + 1],
                        encn_t[st][:, hc * 512:(hc + 1) * 512],
                        start=(st == 0), stop=(st == ST - 1),
                    )
                nc.vector.tensor_copy(
                    gpay[0:1, SL + hc * 512:SL + (hc + 1) * 512], cp_ps[:]
                )

            # ---- the one AllGather ----
            g_in = dram_p.tile([NG], dt.float32)
            nc.sync.dma_start(g_in[:], gpay[:])
            g_out = dram_p.tile([NCORES * NG], dt.float32)
            nc.gpsimd.collective_compute(
                "AllGather", mybir.AluOpType.bypass,
                replica_groups=[list(range(NCORES))],
                ins=[g_in[:].opt()], outs=[g_out[:].opt()],
            )
            g_view = g_out[:].rearrange("(r c) -> r c", c=NG)

            # ---- global combine (identical on every core) ----
            mz = small_p.tile([1, 2 * NCORES], dt.float32)  # r-major (m, Z) pairs
            nc.sync.dma_start(mz[:], g_view[:, SL + H:SL + H + 2])
            mzv = mz[:].rearrange("o (r c) -> o r c", c=2)
            m_row = mzv[:, :, 0:1]
            z_row = mzv[:, :, 1:2]
            M = small_p.tile([1, 1], dt.float32)
            nc.vector.reduce_max(M[:], m_row, axis=AX.XY)
            negM = small_p.tile([1, 1], dt.float32)
            nc.vector.tensor_scalar_mul(negM[:], M[:], -1.0)
            w_row = small_p.tile([1, NCORES], dt.float32)
            nc.scalar.activation(
                w_row[:], m_row, AF.Exp, bias=negM[:], scale=1.0
            )
            zz = small_p.tile([1, NCORES], dt.float32)
            nc.vector.tensor_mul(zz[:], w_row[:], z_row)
            Zg = small_p.tile([1, 1], dt.float32)
            nc.vector.reduce_sum(Zg[:], zz[:], axis=AX.X)
            rZg = small_p.tile([1, 1], dt.float32)
            nc.vector.reciprocal(rZg[:], Zg[:])
            w8 = small_p.tile([1, NCORES], dt.float32)
            nc.vector.tensor_scalar_mul(w8[:], w_row[:], rZg[:])

            # broadcast negM / rZg to 128 partitions via K=1 f32 matmul
            bc_in = small_p.tile([1, 2], dt.float32)
            nc.vector.tensor_copy(bc_in[0:1, 0:1], negM[:])
            nc.vector.tensor_copy(bc_in[0:1, 1:2], rZg[:])
            bc_ps = tail_p.tile([P, 2], dt.float32, tag="tail")
            nc.tensor.matmul(bc_ps[:], ones_sb[:], bc_in[:], start=True, stop=True)
            bc = small_p.tile([P, 2], dt.float32)
            nc.vector.tensor_copy(bc[:], bc_ps[:])

            # w8 as a column (8 partitions) via K=1 f32 matmul
            w8c_ps = tail_p.tile([NCORES, 1], dt.float32, tag="tail")
            nc.tensor.matmul(
                w8c_ps[:], w8[:], ones_sb[0:1, 0:1], start=True, stop=True
            )
            w8c = small_p.tile([NCORES, 1], dt.float32)
            nc.vector.tensor_copy(w8c[:], w8c_ps[:])

            # context = sum_i w8[i] * cpart_i  (f32 matmul, K=8)
            cpm = small_p.tile([NCORES, H], dt.float32)
            nc.sync.dma_start(cpm[:], g_view[:, SL:SL + H])
            ctx_sb = small_p.tile([1, H], dt.float32)
            for hc in range(4):
                cx_ps = tail_p.tile([1, 512], dt.float32, tag="tail")
                nc.tensor.matmul(
                    cx_ps[:], w8c[:], cpm[:, hc * 512:(hc + 1) * 512],
                    start=True, stop=True,
                )
                nc.vector.tensor_copy(ctx_sb[0:1, hc * 512:(hc + 1) * 512], cx_ps[:])
            nc.sync.dma_start(ctx_d[:], ctx_sb[:])

            # attn = exp(scores - M) / Zg over all S, laid out [128, 32]
            scr = small_p.tile([P, S // P], dt.float32)
            nc.sync.dma_start(
                scr[:, :],
                g_view[:, 0:SL].rearrange("r (p j) -> r p j", j=S // P),
            )
            p_all = small_p.tile([P, S // P], dt.float32)
            nc.scalar.activation(
                p_all[:], scr[:], AF.Exp, bias=bc[:, 0:1], scale=1.0
            )
            attn_sb = small_p.tile([P, S // P], dt.float32)
            nc.vector.tensor_scalar_mul(attn_sb[:], p_all[:], bc[:, 1:2])
            nc.sync.dma_start(
                attn_d[:].rearrange("(p j) -> p j", j=S // P), attn_sb[:]
            )

    nc.finalize()
    return nc


def _get_nc():
    if "nc" not in _CACHE:
        _CACHE["nc"] = _build()
    return _CACHE["nc"]


def _execute(inputs, trace=False):
    from concourse.bass_utils import run_bass_kernel_spmd

    nc = _get_nc()
    dh = np.ascontiguousarray(
        np.asarray(inputs["decoder_hidden"], dtype=np.float32).reshape(H)
    )
    enc = np.ascontiguousarray(
        np.asarray(inputs["encoder_outputs"], dtype=np.float32).reshape(S, H)
    )
    w_h = np.ascontiguousarray(np.asarray(inputs["W_h"], dtype=np.float32))
    w_e = np.ascontiguousarray(np.asarray(inputs["W_e"], dtype=np.float32))
    bb = np.ascontiguousarray(
        np.asarray(inputs["b"], dtype=np.float32).reshape(H)
    )
    vv = np.ascontiguousarray(
        np.asarray(inputs["v"], dtype=np.float32).reshape(H)
    )
    ident = np.eye(P, dtype=ml_dtypes.bfloat16)
    ones = np.ones((1, P), dtype=np.float32)

    in_maps = []
    for i in range(NCORES):
        in_maps.append({
            "enc": enc[i * SL:(i + 1) * SL],
            "w_e": w_e,
            "w_h": w_h[i * WHL:(i + 1) * WHL],
            "dh": dh,
            "vv": vv,
            "bb": bb,
            "ident": ident,
            "ones": ones,
        })

    res = run_bass_kernel_spmd(nc, in_maps, core_ids=list(range(NCORES)),
                               trace=trace)
    out = res.results[0]
    context = np.asarray(out["ctx"], dtype=np.float32).reshape(H, 1)
    attn = np.asarray(out["attn"], dtype=np.float32).reshape(S)
    return (context, attn), res


def kernel(**inputs):
    (context, attn), _ = _execute(inputs, trace=False)
    return context, attn
